# revision 9
# baseline (speedup 1.0000x reference)
"""Chamfer distance kernel v2 for Trainium2 (Bass/Tile), SPMD over 8 NeuronCores.

Changes vs v1 (195.7us):
  - fp16 PE transposes (identity fp16, fp16 psum) — halves transpose time.
  - Chunked phase 0: x/y loaded in 2 chunks each, per-chunk convert/square/
    transpose chains so the first main matmul starts ~10us earlier.
  - Row path folds only to 1024 wide on DVE; the [128,1024] partial mins are
    DMA'd out per n-tile and the final row min happens on the host. Saves the
    last two folds + tensor_reduce (~27us of DVE).
  - Squares/x2/y2 computed on fp16 copies (DVE 2x/4x modes).
"""

import sys

if "/opt/trn_rl_repo" not in sys.path:
    sys.path.insert(0, "/opt/trn_rl_repo")

import numpy as np

B = 8
N = 4096
M = 4096
K = 64
NT = 128
MT = 512
KA = K + 1

_COMPILED = {}
LAST_RESULTS = None

FOLD_OUT = 1024  # row partial-min width shipped to host


def _build(n_rows, m_cols, num_cores):
    import concourse.bacc as bacc
    import concourse.mybir as mybir
    import concourse.tile as tile
    from concourse.masks import make_identity

    f32 = mybir.dt.float32
    f16 = mybir.dt.float16
    u32 = mybir.dt.uint32
    AX = mybir.AxisListType
    OP = mybir.AluOpType

    n_nt = n_rows // NT          # 32 n-tiles
    n_ch = 2                     # chunks per operand (points 0:2048, 2048:4096)
    CH = n_rows // n_ch          # 2048 points per chunk
    CHW = CH // 2                # 1024 sbuf cols per chunk (r-major packing)

    nc = bacc.Bacc(
        "TRN2", target_bir_lowering=False, debug=False, num_devices=num_cores
    )
    xd = nc.dram_tensor("x", [n_rows, K], f32, kind="ExternalInput")
    yd = nc.dram_tensor("y", [m_cols, K], f32, kind="ExternalInput")
    rowp = nc.dram_tensor("rowp", [n_rows, FOLD_OUT], f16, kind="ExternalOutput")
    outc = nc.dram_tensor("outc", [128, m_cols], f16, kind="ExternalOutput")

    with tile.TileContext(nc) as tc:
        with (
            tc.tile_pool(name="const", bufs=1) as cpool,
            tc.tile_pool(name="mpsum", bufs=2, space="PSUM") as ps_pool,
            tc.tile_pool(name="tsbp", bufs=4) as tsb_pool,
            tc.tile_pool(name="scrp", bufs=3) as scr_pool,
            tc.tile_pool(name="work", bufs=2) as wpool,
        ):
            ident32 = cpool.tile([128, 128], f32, name="ident32")
            make_identity(nc, ident32)
            ident16 = cpool.tile([128, 128], f16, name="ident16")
            nc.vector.tensor_scalar_mul(ident16, ident32, 1.0)

            x2t = cpool.tile([128, n_nt], f32, name="x2t")
            y2t16 = cpool.tile([128, n_nt], f16, name="y2t16")
            y2r0 = cpool.tile([16, 128], f16, name="y2r0")
            y2r1 = cpool.tile([16, 128], f16, name="y2r1")

            xt_parts = [
                cpool.tile([KA, CH], f16, name=f"xtp{i}") for i in range(n_ch)
            ]
            yt_parts = [
                cpool.tile([KA, CH], f16, name=f"ytp{i}") for i in range(n_ch)
            ]

            ONE2 = 0x3C003C00  # two packed fp16 1.0s

            # per-chunk load -> fp16 convert -> squares -> reduce; the fp16
            # copy also feeds the PE transposes that build the K-major parts.
            def load_chunk(dram, i, name):
                sb = cpool.tile([128, CHW], f32, name=f"{name}sb{i}")
                nc.sync.dma_start(
                    sb,
                    dram[:].rearrange("(p r) k -> p (r k)", p=128)[
                        :, i * CHW : (i + 1) * CHW
                    ],
                )
                sb16 = cpool.tile([128, CHW], f16, name=f"{name}16_{i}")
                nc.vector.tensor_scalar_mul(sb16, sb, 1.0)
                sq = wpool.tile([128, CHW], f16, tag="sq", name="sq")
                nc.vector.tensor_tensor(sq, sb16, sb16, OP.mult)
                return sb16, sq

            def build_part(sb16, part, scale, i, y2row=None):
                # 16 transposes [128,64] -> [64,128] fp16 into one psum tile,
                # drained in two halves so the first matmuls can start after
                # only 8 transposes (subtile deps track the psum ranges).
                if y2row is None:
                    nc.gpsimd.memset(part[K : K + 1, :].bitcast(u32), ONE2)
                tp = ps_pool.tile([128, 2 * CH], f16, tag="ps", name="tp")
                HC = CH // 2
                for s in range(2):
                    for j in range(s * 8, s * 8 + 8):
                        nc.tensor.transpose(
                            tp[0:K, j * 128 : (j + 1) * 128],
                            sb16[:, j * K : (j + 1) * K],
                            ident16,
                        )
                    dst = part[0:K, s * HC : (s + 1) * HC]
                    srcp = tp[0:K, s * HC : (s + 1) * HC]
                    nc.vector.tensor_scalar_mul(dst, srcp, scale)
                    if y2row is not None:
                        nc.sync.dma_start(
                            part[K : K + 1, s * HC : (s + 1) * HC],
                            y2row[s * 8 : (s + 1) * 8, :],
                        )

            # ---- chunk 0 of both operands ----
            # One shared psum staging tile (waves A/B) so the main loop's
            # first ps tile gets the second pool buffer with no reuse-wait.
            # x chunk 0 loads on the Activation hwdge queue so both input
            # transfers run in parallel.
            ysb0 = cpool.tile([128, CHW], f32, name="ysb0")
            nc.sync.dma_start(
                ysb0, yd[:].rearrange("(p r) k -> p (r k)", p=128)[:, 0:CHW]
            )
            xsb0 = cpool.tile([128, CHW], f32, name="xsb0")
            nc.scalar.dma_start(
                xsb0, xd[:].rearrange("(p r) k -> p (r k)", p=128)[:, 0:CHW]
            )
            ysb16_0 = cpool.tile([128, CHW], f16, name="y16_0")
            nc.vector.tensor_scalar_mul(ysb16_0, ysb0, 1.0)
            xsb16_0 = cpool.tile([128, CHW], f16, name="x16_0")
            nc.vector.tensor_scalar_mul(xsb16_0, xsb0, 1.0)
            ysq0 = wpool.tile([128, CHW], f16, tag="sq", name="ysq0")
            nc.vector.tensor_tensor(ysq0, ysb16_0, ysb16_0, OP.mult)
            with nc.allow_low_precision(reason="y2 is fp16 in the matmul anyway"):
                nc.vector.tensor_reduce(
                    y2t16[:, 0:16],
                    ysq0.rearrange("p (t k) -> p t k", k=K),
                    AX.X,
                    OP.add,
                )

            HC = CH // 2
            yt0, xt0 = yt_parts[0], xt_parts[0]
            nc.gpsimd.memset(xt0[K : K + 1, :].bitcast(u32), ONE2)
            tp0 = ps_pool.tile([128, 2 * CH], f16, tag="ps", name="tp0")
            # wave A: y half-0 -> cols 0:1024, x half-0 -> cols 1024:2048,
            # y2 mini-transpose -> partitions 64:80 (free in every wave).
            for j in range(8):
                nc.tensor.transpose(
                    tp0[0:K, j * 128 : (j + 1) * 128],
                    ysb16_0[:, j * K : (j + 1) * K],
                    ident16,
                )
            for j in range(8):
                nc.tensor.transpose(
                    tp0[0:K, HC + j * 128 : HC + (j + 1) * 128],
                    xsb16_0[:, j * K : (j + 1) * K],
                    ident16,
                )
            nc.tensor.transpose(tp0[64:80, 0:128], y2t16[:, 0:16], ident16)
            nc.scalar.copy(y2r0, tp0[64:80, 0:128])
            nc.vector.tensor_scalar_mul(yt0[0:K, 0:HC], tp0[0:K, 0:HC], 1.0)
            nc.sync.dma_start(yt0[K : K + 1, 0:HC], y2r0[0:8, :])
            nc.vector.tensor_scalar_mul(xt0[0:K, 0:HC], tp0[0:K, HC:CH], -2.0)
            # x2 for chunk 0 (gates the first drain, not the first matmul)
            xsq0 = wpool.tile([128, CHW], f16, tag="sq", name="xsq0")
            nc.vector.tensor_tensor(xsq0, xsb16_0, xsb16_0, OP.mult)
            nc.vector.tensor_reduce(
                x2t[:, 0:16],
                xsq0.rearrange("p (t k) -> p t k", k=K),
                AX.X,
                OP.add,
            )
            # wave B: the half-1s -> cols 2048:4096
            for j in range(8, 16):
                nc.tensor.transpose(
                    tp0[0:K, CH + (j - 8) * 128 : CH + (j - 7) * 128],
                    ysb16_0[:, j * K : (j + 1) * K],
                    ident16,
                )
            for j in range(8, 16):
                nc.tensor.transpose(
                    tp0[0:K, 3 * HC + (j - 8) * 128 : 3 * HC + (j - 7) * 128],
                    xsb16_0[:, j * K : (j + 1) * K],
                    ident16,
                )
            nc.vector.tensor_scalar_mul(
                yt0[0:K, HC:CH], tp0[0:K, CH : 3 * HC], 1.0
            )
            nc.sync.dma_start(yt0[K : K + 1, HC:CH], y2r0[8:16, :])
            nc.vector.tensor_scalar_mul(
                xt0[0:K, HC:CH], tp0[0:K, 3 * HC : 4 * HC], -2.0
            )

            # ---- main loop ----
            colacc = cpool.tile([128, m_cols], f16, name="colacc")

            for t in range(n_nt):
                xt = xt_parts[(t * 128) // CH]
                xo = (t * 128) % CH
                x2col = x2t[:, t : t + 1]

                tsb = tsb_pool.tile([128, m_cols], f16, tag="tsb", name="tsb")
                for half in range(2):
                    ps = ps_pool.tile([128, 2048], f32, tag="ps", name="ps")
                    yt = yt_parts[half]
                    for h in range(2048 // MT):
                        nc.tensor.matmul(
                            ps[:, h * MT : (h + 1) * MT],
                            lhsT=xt[:, xo : xo + 128],
                            rhs=yt[:, h * MT : (h + 1) * MT],
                            start=True,
                            stop=True,
                        )
                    nc.scalar.add(
                        tsb[:, half * 2048 : (half + 1) * 2048], ps, x2col
                    )
                    # just-in-time builds: after t=0's first-half matmuls are
                    # queued, build y part 1 (needed by t=0 second half), then
                    # x part 1 (needed at t=16).
                    if t == 0 and half == 0:
                        ysb16_1, ysq1 = load_chunk(yd, 1, "y")
                        with nc.allow_low_precision(reason="y2 is fp16 in the matmul anyway"):
                            nc.vector.tensor_reduce(
                                y2t16[:, 16:32],
                                ysq1.rearrange("p (t k) -> p t k", k=K),
                                AX.X,
                                OP.add,
                            )
                        y2p1 = ps_pool.tile([128, 2 * CH], f16, tag="ps", name="y2p1")
                        nc.tensor.transpose(
                            y2p1[0:16, 0:128], y2t16[:, 16:32], ident16
                        )
                        nc.scalar.copy(y2r1, y2p1[0:16, 0:128])
                        build_part(ysb16_1, yt_parts[1], 1.0, 1, y2row=y2r1)

                if t == 0:
                    xsb16_1, xsq1 = load_chunk(xd, 1, "x")
                    nc.vector.tensor_reduce(
                        x2t[:, 16:32],
                        xsq1.rearrange("p (t k) -> p t k", k=K),
                        AX.X,
                        OP.add,
                    )
                    build_part(xsb16_1, xt_parts[1], -2.0, 1)

                last = t == n_nt - 1

                def row_path():
                    scr = scr_pool.tile([128, 2048], f16, tag="scr", name="scr")
                    nc.vector.tensor_tensor(
                        scr, tsb[:, 0:2048], tsb[:, 2048:4096], OP.min
                    )
                    nc.vector.tensor_tensor(
                        scr[:, 0:FOLD_OUT],
                        scr[:, 0:FOLD_OUT],
                        scr[:, FOLD_OUT : 2 * FOLD_OUT],
                        OP.min,
                    )
                    eng = nc.scalar if t >= n_nt - 4 else nc.sync
                    eng.dma_start(
                        rowp[t * 128 : (t + 1) * 128, :], scr[:, 0:FOLD_OUT]
                    )

                if last:
                    # row first, then quartered col update with the writeback
                    # DMA launched per quarter so it overlaps the tail compute
                    row_path()
                    q = m_cols // 4
                    for j in range(4):
                        nc.vector.tensor_tensor(
                            colacc[:, j * q : (j + 1) * q],
                            tsb[:, j * q : (j + 1) * q],
                            colacc[:, j * q : (j + 1) * q],
                            OP.min,
                        )
                        qeng = nc.sync if j % 2 == 0 else nc.scalar
                        qeng.dma_start(
                            outc[:, j * q : (j + 1) * q],
                            colacc[:, j * q : (j + 1) * q],
                        )
                else:
                    if t == 0:
                        nc.vector.tensor_copy(colacc, tsb)
                    else:
                        nc.vector.tensor_tensor(colacc, tsb, colacc, OP.min)
                    row_path()

    nc.compile()
    return nc


def _get(n_rows, m_cols, num_cores):
    key = (n_rows, m_cols, num_cores)
    if key not in _COMPILED:
        _COMPILED[key] = _build(n_rows, m_cols, num_cores)
    return _COMPILED[key]


def _run(x, y, n_rows, m_cols, num_cores, trace=False):
    global LAST_RESULTS
    from concourse import bass_utils

    nc = _get(n_rows, m_cols, num_cores)
    in_maps = [
        {"x": np.ascontiguousarray(x[b]), "y": np.ascontiguousarray(y[b])}
        for b in range(num_cores)
    ]
    res = bass_utils.run_bass_kernel_spmd(
        nc, in_maps, core_ids=list(range(num_cores)), trace=trace
    )
    LAST_RESULTS = res
    return [(r["rowp"], r["outc"]) for r in res.results]


def _postprocess(outs):
    total = 0.0
    for rowpart, colacc in outs:
        rmin = rowpart.astype(np.float32).min(axis=1)
        colmin = colacc.astype(np.float32).min(axis=0)
        d1 = np.sqrt(np.maximum(rmin.astype(np.float64), 0.0)).mean()
        d0 = np.sqrt(np.maximum(colmin.astype(np.float64), 0.0)).mean()
        total += d0 + d1
    return np.float32(total / len(outs))


def kernel(input1, input2):
    x = np.asarray(input1, dtype=np.float32)
    y = np.asarray(input2, dtype=np.float32)
    assert x.shape == (B, N, K) and y.shape == (B, M, K), (x.shape, y.shape)
    outs = _run(x, y, N, M, B)
    return _postprocess(outs)
